# revision 1
# baseline (speedup 1.0000x reference)
"""Trainium2 Bass kernel for nn_CustomizableLRCLLoss.

Math restructure
----------------
The reference enumerates the P = N(N-1)/2 upper-triangle pairs per batch row
and computes, per pair (i, j):

    dr   = r_i - r_j,  t = sign(dr),  ds = s_i - s_j
    tau  = sum_k ct_k * softplus(a_k*|dr| + b_k)
    m    = tau - t*ds
    g    = sum_k cg_k * softplus(a_k*m + b_k)
    w    = FLOOR + sum_k cw_k * sigmoid(a6_k*|dr| + b6_k)
    loss = g*w over kept pairs (dr != 0), row-averaged, then batch-averaged.

The pair value is invariant under (i,j) -> (j,i), so we can evaluate any
orientation.  We cover the i<j triangle as:
  * 15 "rect" segments: i-block t (32 i's), j in [32(t+1), 512)  -> each
    cross-block pair exactly once
  * one "band" pass: 16 diagonal 32x32 blocks -> each in-block pair twice
    plus the diagonal (dr == 0 there, handled by the Z-count correction)
Pairs with dr == 0 contribute exactly L0 = g(tau0)*w(0), a constant we
compute on the host, so instead of masking we subtract Z*L0.

Sharding: data-parallel over batch rows, 4 rows per core x 8 cores.
Per-core partition layout: p = 32*b_loc + ii  (4 local rows x 32 i's);
i = 32*t + ii.  All per-pair tensors live as [128, 4352] SBUF tiles
(3840 rect cols + 512 band cols), processed by full-width instructions.

sign(dr)*ds is computed without a Sign activation by XOR-ing dr's sign bit
onto ds (exact except for the measure-zero off-diagonal tie case, whose
final-loss impact is ~1e-7 relative).
"""

import numpy as np
from contextlib import ExitStack

import concourse.bass as bass
import concourse.mybir as mybir
import concourse.tile as tile
from concourse.bass_utils import run_bass_kernel_spmd

F32 = mybir.dt.float32
U32 = mybir.dt.uint32
AF = mybir.ActivationFunctionType
OP = mybir.AluOpType

B, N = 32, 512
NCORES = 8
BLOC = B // NCORES          # 4 batch rows per core
NBLK, BI = 16, 32           # 16 i-blocks of 32
NPART = BLOC * BI           # 128 partitions
RECT_W = [N - BI * (t + 1) for t in range(NBLK - 1)]   # 480, 448, ..., 32
RECT_OFF = np.concatenate([[0], np.cumsum(RECT_W)]).tolist()
RTOT = int(sum(RECT_W))     # 3840
FTOT = RTOT + N             # 4352 (rects + band)
FLOOR = 0.001
EPS = 1e-6
MC = 20.0               # g-chain m clamp; exp(4*MC+2) stays finite

A8 = np.linspace(0.5, 4.0, 8)
B8 = np.linspace(-2.0, 2.0, 8)
A6 = np.linspace(0.5, 4.0, 6)
B6 = np.linspace(-2.0, 2.0, 6)


def _softplus(x):
    return np.log1p(np.exp(-np.abs(x))) + np.maximum(x, 0.0)


def _sigmoid(x):
    return 1.0 / (1.0 + np.exp(-x))


def _build(ct, cg, cw, l0, reps=1):
    """Build the per-core Bass program (same program on all 8 cores)."""
    nc = bass.Bass()
    pred = nc.dram_tensor("pred4", [BLOC, N], F32, kind="ExternalInput")
    tg = nc.dram_tensor("tg4", [BLOC, N], F32, kind="ExternalInput")
    out = nc.dram_tensor("out4", [BLOC, 1], F32, kind="ExternalOutput")
    _emit(nc, pred, tg, out, ct, cg, cw, l0, reps)
    return nc


def _emit(nc, pred, tg, out, ct, cg, cw, l0, reps=1):
    scratch = nc.dram_tensor("rstd_scratch", [BLOC], F32, kind="Internal")

    with tile.TileContext(nc) as tc, ExitStack() as ctx:
        singles = ctx.enter_context(tc.tile_pool(name="singles", bufs=1))
        big = ctx.enter_context(tc.tile_pool(name="big", bufs=1))
        sp_pool = ctx.enter_context(tc.tile_pool(name="sp", bufs=4))
        psum = ctx.enter_context(tc.tile_pool(name="psum", bufs=1, space="PSUM"))

        # activation() requires bias as a [P,1] AP; build a constants tile.
        bias_vals = list(B8) + list(B6) + [EPS, 0.0, 1.0] + list(-B8)
        biases = singles.tile([NPART, len(bias_vals)], F32)
        for i, v in enumerate(bias_vals):
            nc.vector.memset(biases[:, i:i + 1], float(v))
        b8_ap = lambda k, p=NPART: biases[:p, k:k + 1]
        b6_ap = lambda k, p=NPART: biases[:p, 8 + k:9 + k]
        eps_ap = lambda p: biases[:p, 14:15]
        zero_ap = lambda p: biases[:p, 15:16]
        one_ap = lambda p: biases[:p, 16:17]
        nb8_ap = lambda k, p=NPART: biases[:p, 17 + k:18 + k]

        # ---- per-row stats: rstd = 1/sqrt(var + eps) -------------------
        pred_rows = singles.tile([BLOC, N], F32)
        nc.sync.dma_start(out=pred_rows[:, :], in_=pred[:, :])
        sum4 = singles.tile([BLOC, 1], F32)
        nc.vector.reduce_sum(out=sum4[:, :], in_=pred_rows[:, :],
                             axis=mybir.AxisListType.X)
        mean4 = singles.tile([BLOC, 1], F32)
        nc.vector.tensor_scalar(out=mean4[:, :], in0=sum4[:, :],
                                scalar1=1.0 / N, scalar2=None, op0=OP.mult)
        xm = singles.tile([BLOC, N], F32)
        nc.vector.tensor_scalar(out=xm[:, :], in0=pred_rows[:, :],
                                scalar1=mean4[:, 0:1], scalar2=None,
                                op0=OP.subtract)
        xmsq = singles.tile([BLOC, N], F32)
        ssum = singles.tile([BLOC, 1], F32)
        nc.vector.tensor_tensor(out=xmsq[:, :], in0=xm[:, :], in1=xm[:, :],
                                op=OP.mult)
        nc.vector.reduce_sum(out=ssum[:, :], in_=xmsq[:, :],
                             axis=mybir.AxisListType.X)
        # rstd = exp(-0.5 * ln(ssum/N + eps))   (Ln/Exp share one table set)
        lnv = singles.tile([BLOC, 1], F32)
        nc.scalar.activation(out=lnv[:, :], in_=ssum[:, :], func=AF.Ln,
                             bias=eps_ap(BLOC), scale=1.0 / N)
        rstd4 = singles.tile([BLOC, 1], F32)
        nc.scalar.activation(out=rstd4[:, :], in_=lnv[:, :], func=AF.Exp,
                             bias=zero_ap(BLOC), scale=-0.5)
        nc.sync.dma_start(out=scratch[:], in_=rstd4[:, 0:1])

        # ---- broadcast / column loads ----------------------------------
        def dram_ap(handle, ap, off=0):
            a = handle[:, :] if len(handle.shape) > 1 else handle[:]
            return bass.AP(tensor=a.tensor, offset=a.offset + off, ap=ap)

        tg_bc = singles.tile([NPART, N], F32)       # [p=(b,ii), j] = tg[b, j]
        pr_bc = singles.tile([NPART, N], F32)
        tg_col = singles.tile([NPART, NBLK], F32)   # [p, t] = tg[b, 32t+ii]
        pr_col = singles.tile([NPART, NBLK], F32)
        rstd_b = singles.tile([NPART, 1], F32)      # [p] = rstd[b]
        for b in range(BLOC):
            pp = slice(BI * b, BI * (b + 1))
            nc.sync.dma_start(out=tg_bc[pp, :],
                              in_=dram_ap(tg, [[0, BI], [1, N]], off=b * N))
            nc.sync.dma_start(out=pr_bc[pp, :],
                              in_=dram_ap(pred, [[0, BI], [1, N]], off=b * N))
            nc.sync.dma_start(out=tg_col[pp, :],
                              in_=dram_ap(tg, [[1, BI], [BI, NBLK]], off=b * N))
            nc.sync.dma_start(out=pr_col[pp, :],
                              in_=dram_ap(pred, [[1, BI], [BI, NBLK]], off=b * N))
            nc.sync.dma_start(out=rstd_b[pp, :],
                              in_=dram_ap(scratch, [[0, BI], [1, 1]], off=b))

        ps_bc = singles.tile([NPART, N], F32)       # rstd-scaled predictions
        nc.vector.tensor_scalar(out=ps_bc[:, :], in0=pr_bc[:, :],
                                scalar1=rstd_b[:, 0:1], scalar2=None,
                                op0=OP.mult)
        ps_col = singles.tile([NPART, NBLK], F32)
        nc.vector.tensor_scalar(out=ps_col[:, :], in0=pr_col[:, :],
                                scalar1=rstd_b[:, 0:1], scalar2=None,
                                op0=OP.mult)

        for _rep in range(reps):
            # ---- dr / ds over rects + band ---------------------------------
            dr = big.tile([NPART, FTOT], F32)
            ds = big.tile([NPART, FTOT], F32)
            for t in range(NBLK - 1):
                o, w, j0 = RECT_OFF[t], RECT_W[t], BI * (t + 1)
                nc.vector.tensor_scalar(out=dr[:, o:o + w], in0=tg_bc[:, j0:N],
                                        scalar1=tg_col[:, t:t + 1], scalar2=None,
                                        op0=OP.subtract)
                nc.vector.tensor_scalar(out=ds[:, o:o + w], in0=ps_bc[:, j0:N],
                                        scalar1=ps_col[:, t:t + 1], scalar2=None,
                                        op0=OP.subtract)
            band3 = lambda ap: ap.rearrange("p (t j) -> p t j", t=NBLK)
            tgc3 = tg_col[:, :].unsqueeze(2).broadcast_to([NPART, NBLK, BI])
            psc3 = ps_col[:, :].unsqueeze(2).broadcast_to([NPART, NBLK, BI])
            nc.vector.scalar_tensor_tensor(out=band3(dr[:, RTOT:FTOT]),
                                           in0=band3(tg_bc[:, :]), scalar=1.0,
                                           in1=tgc3, op0=OP.mult,
                                           op1=OP.subtract)
            nc.vector.scalar_tensor_tensor(out=band3(ds[:, RTOT:FTOT]),
                                           in0=band3(ps_bc[:, :]), scalar=1.0,
                                           in1=psc3, op0=OP.mult,
                                           op1=OP.subtract)

            # ---- |dr|, sign bits, t*ds, zero counts ------------------------
            absd = big.tile([NPART, FTOT], F32)
            nc.vector.tensor_scalar(out=absd[:, :].bitcast(U32),
                                    in0=dr[:, :].bitcast(U32),
                                    scalar1=0x7FFFFFFF, scalar2=None,
                                    op0=OP.bitwise_and)
            m_acc = big.tile([NPART, FTOT], F32)   # sgn borrows m_acc's slot
            sgn = m_acc[:, :].bitcast(U32)
            nc.vector.tensor_scalar(out=sgn, in0=dr[:, :].bitcast(U32),
                                    scalar1=0x80000000, scalar2=None,
                                    op0=OP.bitwise_and)
            tds = big.tile([NPART, FTOT], F32)          # = sign(dr)*ds exactly
            nc.vector.tensor_tensor(out=tds[:, :].bitcast(U32),
                                    in0=ds[:, :].bitcast(U32), in1=sgn,
                                    op=OP.bitwise_xor)

            partials = singles.tile([NPART, 4], F32)    # num1, z1, num2, z2
            nc.vector.tensor_scalar(out=dr[:, 0:RTOT], in0=absd[:, 0:RTOT],
                                    scalar1=0.0, scalar2=None, op0=OP.is_equal,
                                    op1=OP.add, accum_out=partials[:, 1:2])
            nc.vector.tensor_scalar(out=dr[:, RTOT:FTOT], in0=absd[:, RTOT:FTOT],
                                    scalar1=0.0, scalar2=None, op0=OP.is_equal,
                                    op1=OP.add, accum_out=partials[:, 3:4])

            # ---- w = FLOOR + sum_k cw_k * sigmoid(a6_k*|dr| + b6_k) --------
            w_acc = big.tile([NPART, FTOT], F32)
            for k in range(6):
                sw = sp_pool.tile([NPART, FTOT], F32, tag="sp")
                nc.scalar.activation(out=sw[:, :], in_=absd[:, :], func=AF.Sigmoid,
                                     bias=b6_ap(k), scale=float(A6[k]))
                if k == 0:
                    nc.gpsimd.tensor_scalar(out=w_acc[:, :], in0=sw[:, :],
                                            scalar1=float(cw[0]), scalar2=FLOOR,
                                            op0=OP.mult, op1=OP.add)
                else:
                    nc.vector.scalar_tensor_tensor(out=w_acc[:, :], in0=sw[:, :],
                                                   scalar=float(cw[k]),
                                                   in1=w_acc[:, :], op0=OP.mult,
                                                   op1=OP.add)

            # softplus(z) = ln(1 + exp(z)): no native Softplus table in this
            # toolchain; Exp and Ln share the natural_log_exp table set.
            def softplus_act(in_ap, k):
                e = sp_pool.tile([NPART, FTOT], F32, tag="sp")
                nc.scalar.activation(out=e[:, :], in_=in_ap, func=AF.Exp,
                                     bias=b8_ap(k), scale=float(A8[k]))
                l = sp_pool.tile([NPART, FTOT], F32, tag="sp")
                nc.scalar.activation(out=l[:, :], in_=e[:, :], func=AF.Ln,
                                     bias=one_ap(NPART), scale=1.0)
                return l

            # ---- m = sum_k ct_k * softplus(a8_k*|dr| + b8_k) - t*ds --------
            # tau-chain: exp(a8_k*|dr|) = E^(k+1) with E = exp(|dr|/2); the
            # exp(b8_k) factor folds into the Ln input scale.  E lives in dr
            # (dead after absd/sgn/zcounts); powers alternate ds / pool tiles.
            E = dr                              # dr dead until the num pass
            nc.scalar.activation(out=E[:, :], in_=absd[:, :], func=AF.Exp,
                                 bias=zero_ap(NPART), scale=0.5)
            fk = E[:, :]
            for k in range(8):
                if k > 0:
                    if k % 2 == 1:
                        nxt = ds[:, :]          # ds dead until lin is written
                    else:
                        pw = sp_pool.tile([NPART, FTOT], F32, tag="sp")
                        nxt = pw[:, :]
                    nc.gpsimd.tensor_tensor(out=nxt, in0=fk, in1=E[:, :],
                                            op=OP.mult)
                    fk = nxt
                sp = sp_pool.tile([NPART, FTOT], F32, tag="sp")
                nc.scalar.activation(out=sp[:, :], in_=fk, func=AF.Ln,
                                     bias=one_ap(NPART),
                                     scale=float(np.exp(B8[k])))
                if k == 0:
                    nc.vector.scalar_tensor_tensor(out=m_acc[:, :], in0=sp[:, :],
                                                   scalar=float(ct[0]),
                                                   in1=tds[:, :], op0=OP.mult,
                                                   op1=OP.subtract)
                else:
                    nc.vector.scalar_tensor_tensor(out=m_acc[:, :], in0=sp[:, :],
                                                   scalar=float(ct[k]),
                                                   in1=m_acc[:, :], op0=OP.mult,
                                                   op1=OP.add)

            # ---- g = sum_k cg_k * softplus(a8_k*m + b8_k) ------------------
            # m reaches ~26 and exp(a*m+b) would leave Ln's valid range (2^64),
            # so use softplus(z) = z + softplus(-z):
            #   g = Ag*m + Bg + sum_k cg_k * log1p(exp(-a8_k*m - b8_k))
            # with Ag = sum cg*a8, Bg = sum cg*b8.  exp(-z) <= e^42 after the
            # (practically never active) m >= -10 safety clamp.
            ag = float((np.asarray(cg, np.float64) * A8).sum())
            bg = float((np.asarray(cg, np.float64) * B8).sum())
            nc.vector.tensor_scalar(out=m_acc[:, :], in0=m_acc[:, :],
                                    scalar1=-10.0, scalar2=None, op0=OP.max)
            lin = ds                            # ds is dead after the xor
            nc.vector.tensor_scalar(out=lin[:, :], in0=m_acc[:, :], scalar1=ag,
                                    scalar2=bg, op0=OP.mult, op1=OP.add)
            g_acc = absd                        # absd dead after tau/w/zcounts
            # a8_k = 0.5*(k+1), so exp(-a8_k*m) = F^(k+1) with F = exp(-m/2):
            # one Exp pass + 7 GPSIMD multiplies replaces 8 Exp passes, and
            # exp(-b8_k) folds into the Ln's input scale.  Power tiles rotate
            # through dr and m_acc, both dead here (m_acc after F/lin).
            F = tds                             # tds dead after m-chain seed
            nc.scalar.activation(out=F[:, :], in_=m_acc[:, :], func=AF.Exp,
                                 bias=zero_ap(NPART), scale=-0.5)
            fk = F[:, :]
            for k in range(8):
                if k > 0:
                    nxt = (dr if k % 2 == 1 else m_acc)[:, :]
                    nc.gpsimd.tensor_tensor(out=nxt, in0=fk, in1=F[:, :],
                                            op=OP.mult)
                    fk = nxt
                l = sp_pool.tile([NPART, FTOT], F32, tag="sp")
                nc.scalar.activation(out=l[:, :], in_=fk, func=AF.Ln,
                                     bias=one_ap(NPART),
                                     scale=float(np.exp(-B8[k])))
                nc.vector.scalar_tensor_tensor(
                    out=g_acc[:, :], in0=l[:, :], scalar=float(cg[k]),
                    in1=(lin if k == 0 else g_acc)[:, :], op0=OP.mult, op1=OP.add)

            # ---- num sums: rect and band separately ------------------------
            nc.vector.tensor_tensor(out=dr[:, :], in0=g_acc[:, :],
                                    in1=w_acc[:, :], op=OP.mult)
            nc.vector.reduce_sum(out=partials[:, 0:1], in_=dr[:, 0:RTOT],
                                 axis=mybir.AxisListType.X)
            nc.vector.reduce_sum(out=partials[:, 2:3], in_=dr[:, RTOT:FTOT],
                                 axis=mybir.AxisListType.X)

            # ---- cross-partition reduce (per local row b) via PE -----------
            sel = singles.tile([NPART, NPART], F32)
            nc.vector.memset(sel[:, :], 0.0)
            for b in range(BLOC):
                nc.vector.memset(sel[BI * b:BI * (b + 1), b:b + 1], 1.0)
            mmp = psum.tile([NPART, 4], F32)
            nc.tensor.matmul(out=mmp[:, :], lhsT=sel[:, :], rhs=partials[:, :],
                             start=True, stop=True)
            mm = singles.tile([NPART, 4], F32)
            nc.vector.tensor_copy(out=mm[:, :], in_=mmp[:, :])

            # row_loss = (num1 + num2/2 - L0*(z1 + z2/2)) / (131072 - z1 - z2/2)
            t1 = singles.tile([BLOC, 1], F32)
            nc.vector.scalar_tensor_tensor(out=t1[:, :], in0=mm[0:BLOC, 3:4],
                                           scalar=0.5, in1=mm[0:BLOC, 1:2],
                                           op0=OP.mult, op1=OP.add)
            numt = singles.tile([BLOC, 1], F32)
            nc.vector.scalar_tensor_tensor(out=numt[:, :], in0=mm[0:BLOC, 2:3],
                                           scalar=0.5, in1=mm[0:BLOC, 0:1],
                                           op0=OP.mult, op1=OP.add)
            nc.vector.scalar_tensor_tensor(out=numt[:, :], in0=t1[:, :],
                                           scalar=float(-l0), in1=numt[:, :],
                                           op0=OP.mult, op1=OP.add)
            dent = singles.tile([BLOC, 1], F32)
            nc.vector.tensor_scalar(out=dent[:, :], in0=t1[:, :], scalar1=-1.0,
                                    scalar2=float(N * N / 2.0),
                                    op0=OP.mult, op1=OP.add)
            rden = singles.tile([BLOC, 1], F32)
            nc.vector.reciprocal(out=rden[:, :], in_=dent[:, :])
            rl = singles.tile([BLOC, 1], F32)
            nc.vector.tensor_tensor(out=rl[:, :], in0=numt[:, :], in1=rden[:, :],
                                    op=OP.mult)
            nc.sync.dma_start(out=out[:, :], in_=rl[:, :])

    return out


def _split_multi_waits(nc):
    """This toolchain's walrus encodes at most ONE sync wait per instruction.

    Tile attaches several semaphore waits to a single instruction (body ops
    and the kernel-tail drain).  Split the extras onto same-engine NoOps
    inserted immediately before the instruction: per-engine program order is
    preserved, so sequential waits are equivalent to one multi-wait.
    """
    n = 0
    for f in nc.m.functions:
        for bb in f.blocks:
            new = []
            for inst in bb.instructions:
                si = inst.sync_info
                if si is not None and si.on_wait is not None and len(si.on_wait) > 1:
                    waits = list(si.on_wait)
                    for w in waits[:-1]:
                        n += 1
                        nop = mybir.InstNoOp(name=f"I-splitw-{n}", ins=[], outs=[])
                        nop.engine = inst.engine
                        nop.sync_info = mybir.SyncInfo(on_wait=[w], on_update=[])
                        new.append(nop)
                    si.on_wait = [waits[-1]]
                new.append(inst)
            if n:
                try:
                    bb.instructions[:] = new
                except TypeError:
                    bb.instructions = new
    return nc


def _coeffs(theta_tau, theta_g, theta_w):
    ct = _softplus(np.asarray(theta_tau, np.float64))
    cg = _softplus(np.asarray(theta_g, np.float64))
    cw = _softplus(np.asarray(theta_w, np.float64))
    tau0 = float((ct * _softplus(B8)).sum())
    g0 = float((cg * _softplus(A8 * tau0 + B8)).sum())
    w0 = FLOOR + float((cw * _sigmoid(B6)).sum())
    return ct, cg, cw, g0 * w0



# ---- NEFF disk cache: compiles take minutes; key on the BIR content ----
_NEFF_CACHE_DIR = "/tmp/lrcl_neff_cache"


def _install_neff_cache():
    import hashlib
    import os
    import shutil
    import concourse.bass2jax as bass2jax

    if getattr(bass2jax, "_lrcl_neff_cache", False):
        return
    orig = bass2jax.compile_bir_kernel

    def cached(bir_json, tmpdir, neff_name="file.neff"):
        h = hashlib.sha256(bir_json).hexdigest()[:32]
        cpath = os.path.join(_NEFF_CACHE_DIR, h + ".neff")
        if os.path.exists(cpath):
            dst = os.path.join(tmpdir, neff_name)
            shutil.copy(cpath, dst)
            return dst
        p = orig(bir_json, tmpdir, neff_name)
        try:
            os.makedirs(_NEFF_CACHE_DIR, exist_ok=True)
            tmp = cpath + ".tmp"
            shutil.copy(p, tmp)
            os.replace(tmp, cpath)
        except OSError:
            pass
        return p

    bass2jax.compile_bir_kernel = cached
    bass2jax._lrcl_neff_cache = True


_CACHE = {}


def kernel(predictions, targets, theta_tau, theta_g, theta_w):
    predictions = np.ascontiguousarray(predictions, np.float32)
    targets = np.ascontiguousarray(targets, np.float32)
    ct, cg, cw, l0 = _coeffs(theta_tau, theta_g, theta_w)

    _install_neff_cache()
    key = (ct.tobytes(), cg.tobytes(), cw.tobytes())
    if key not in _CACHE:
        _CACHE[key] = _split_multi_waits(_build(ct, cg, cw, l0))
    nc = _CACHE[key]

    in_maps = [
        {
            "pred4": predictions[c * BLOC:(c + 1) * BLOC],
            "tg4": targets[c * BLOC:(c + 1) * BLOC],
        }
        for c in range(NCORES)
    ]
    res = run_bass_kernel_spmd(nc, in_maps, list(range(NCORES)))
    total = sum(float(res.results[c]["out4"].sum()) for c in range(NCORES))
    return np.asarray(total / B, dtype=np.float32)

